# revision 19
# baseline (speedup 1.0000x reference)
"""Trainium2 Bass kernel for the GWNN2 GNN (4-graph GraphConv x2 + MLP).

V3 strategy (8 NeuronCores, dst-sharded):
  * deg_out[src] and deg_in[dst] both folded into the per-edge weight
    host-side -> each layer's gather table is ONE shared [50176, 128]
    bf16 matrix (q1 = x@W1, q2 = h2@W2); 4x smaller collectives.
  * per-(g, win, half) chunk counts K exact (max over the 8 cores so one
    SPMD NEFF serves all); dma_gather in 8-chunk (1024-desc) sub-calls
    (the SWDGE ring caps a call at ~1024 descriptors).
  * one-hot masks built in chunk-MINOR layout [e, win, c] with two bulk
    DVE tensor_tensor ops per (g, half, win) whose operands are all
    2-byte packed (DVE 2x mode): (md == iota) * w.  The aggregation
    matmul reads the mask chunk column with a strided rhs AP.
  * dense layers (l1, l2, W2 / l3) batched over 4-window groups
    (rhs 512 wide) to amortize PE ldweights.
  * AllGather shard tables between phases.
"""
import sys
import types
from dataclasses import dataclass

if "/opt/trn_rl_repo" not in sys.path:
    sys.path.insert(0, "/opt/trn_rl_repo")

import numpy as np
import ml_dtypes

import concourse.bass as bass
import concourse.bacc as bacc
import concourse.mybir as mybir
import concourse.tile as tile
from concourse.masks import make_identity

BF16 = ml_dtypes.bfloat16
P = 128
KFIX = 10            # mask layout c-pitch (>= max chunks per bucket)


def _install_ntff_hook():
    """Make trace=True usable under axon (antenv.axon_hooks may be absent)."""
    try:
        import antenv
        if "antenv.axon_hooks" in sys.modules:
            return
        m = types.ModuleType("antenv.axon_hooks")
        box = [None]
        m.set_axon_ntff_profile_hook = lambda h: box.__setitem__(0, h)
        m.get_axon_ntff_profile_hook = lambda: box[0]
        sys.modules["antenv.axon_hooks"] = m
        antenv.axon_hooks = m
        try:
            from trn_agent_boot.trn_boot import _ntff_profile_via_ctypes
            hook = _ntff_profile_via_ctypes("/opt/axon/libaxon_pjrt.so")
            if hook is not None:
                m.set_axon_ntff_profile_hook(hook)
        except Exception:
            pass
    except Exception:
        pass


@dataclass
class Cfg:
    n_nodes: int = 50000
    g_num: int = 4
    in_feats: int = 256
    h_feats: int = 128
    n_classes: int = 40
    n_cores: int = 8
    win: int = 128
    win_batch: int = 12         # windows per ft/gather batch
    grp: int = 4                # windows per dense-layer matmul group

    @property
    def shard(self):
        return self.n_nodes // self.n_cores

    @property
    def shard_p(self):
        return ((self.shard + P - 1) // P) * P

    @property
    def rows(self):
        return self.shard_p * self.n_cores

    @property
    def half(self):
        return self.rows // 2

    @property
    def nwin(self):
        return self.shard_p // self.win

    @property
    def cat(self):
        return self.h_feats * self.g_num

    @property
    def kc_cat(self):
        return self.cat // P

    @property
    def kc_in(self):
        return self.in_feats // P

    @property
    def ntile_own(self):
        return self.shard_p // P


def _prep_inputs(cfg: Cfg, in_feat, src, dst, w, W1, W2, l1w, l1b, l2w, l2b,
                 l3w, l3b):
    """Host-side sharding/packing."""
    N, G = cfg.n_nodes, cfg.g_num
    SH, SHP = cfg.shard, cfg.shard_p
    NW, WIN = cfg.nwin, cfg.win
    HALF = cfg.half
    src = np.asarray(src).astype(np.int64)
    dst = np.asarray(dst).astype(np.int64)
    w = np.asarray(w, dtype=np.float32)
    in_feat = np.asarray(in_feat, dtype=np.float32)

    deg_out = np.empty((G, N), np.float32)
    deg_in = np.empty((G, N), np.float32)
    for g in range(G):
        deg_out[g] = np.clip(np.bincount(src[g], minlength=N), 1.0, None) ** -0.5
        deg_in[g] = np.clip(np.bincount(dst[g], minlength=N), 1.0, None) ** -0.5

    src_pad = (src // SH) * SHP + (src % SH)
    half_flag = (src_pad >= HALF).astype(np.int64)
    idx_local = (src_pad - half_flag * HALF).astype(np.int64)

    core_of = dst // SH
    dst_loc = dst % SH
    win_of = dst_loc // WIN
    dst_in_win = (dst_loc % WIN).astype(np.float32)

    w_eff = np.empty((G, src.shape[1]), np.float32)
    for g in range(G):
        w_eff[g] = w[g] * deg_in[g][dst[g]] * deg_out[g][src[g]]

    cnt = np.zeros((cfg.n_cores, G, NW, 2), np.int64)
    for i in range(cfg.n_cores):
        for g in range(G):
            m = core_of[g] == i
            key = win_of[g][m] * 2 + half_flag[g][m]
            cnt[i, g] = np.bincount(key, minlength=NW * 2).reshape(NW, 2)
    K = np.ceil(cnt.max(axis=0) / P).astype(np.int64)       # (G, NW, 2)
    assert K.max() <= KFIX, f"KFIX too small: {K.max()}"
    CHOFF = np.zeros((G, 2, NW + 1), np.int64)
    for g in range(G):
        for h in range(2):
            CHOFF[g, h, 1:] = np.cumsum(K[g, :, h])
    TOTCH = CHOFF[:, :, -1]                                  # (G, 2)

    xpad = np.zeros((cfg.rows, cfg.in_feats), np.float32)
    for i in range(cfg.n_cores):
        xpad[i * SHP:i * SHP + SH] = in_feat[i * SH:(i + 1) * SH]

    def pack_lhsT(W, kc):
        Wr = np.asarray(W, np.float32).reshape(kc, P, -1)
        return np.ascontiguousarray(Wr.transpose(1, 0, 2)).reshape(P, -1)

    W1c = pack_lhsT(W1, cfg.kc_in).astype(BF16)
    W2c = pack_lhsT(W2, cfg.kc_cat).astype(BF16)
    l1wc = pack_lhsT(l1w, cfg.kc_cat).astype(BF16)
    l2wc = pack_lhsT(l2w, cfg.kc_cat).astype(BF16)
    l3wc = pack_lhsT(l3w, cfg.kc_cat).astype(BF16)
    l1bc = np.ascontiguousarray(
        np.asarray(l1b, np.float32).reshape(cfg.kc_cat, P).T)
    l2bc = np.ascontiguousarray(
        np.asarray(l2b, np.float32).reshape(cfg.kc_cat, P).T)
    l3bc = np.asarray(l3b, np.float32).reshape(cfg.n_classes, 1)

    # iota tile for mask build: [128, WIN, KFIX] bf16, value = j
    iota_t = np.ascontiguousarray(
        np.broadcast_to(np.arange(WIN, dtype=np.float32)[None, :, None],
                        (P, WIN, KFIX))).reshape(P, WIN * KFIX).astype(BF16)

    in_maps = []
    for i in range(cfg.n_cores):
        im = {"w1c": W1c, "w2c": W2c, "l1wc": l1wc, "l2wc": l2wc,
              "l3wc": l3wc, "l1bc": l1bc, "l2bc": l2bc, "l3bc": l3bc,
              "iota": iota_t}
        xsh = xpad[i * SHP:(i + 1) * SHP].reshape(
            cfg.ntile_own, P, cfg.kc_in, P)
        im["xtiles"] = np.ascontiguousarray(
            xsh.transpose(0, 3, 2, 1)).reshape(
            cfg.ntile_own, P, cfg.kc_in * P).astype(BF16)

        for g in range(G):
            m = core_of[g] == i
            key = win_of[g][m] * 2 + half_flag[g][m]
            order = np.argsort(key, kind="stable")
            skey = key[order]
            bc = np.bincount(skey, minlength=NW * 2)
            starts = np.concatenate([[0], np.cumsum(bc)[:-1]])
            slot = np.arange(len(skey)) - starts[skey]
            il = idx_local[g][m][order]
            dw = dst_in_win[g][m][order]
            we = w_eff[g][m][order]
            swin = skey // 2
            shf = skey % 2
            for h in range(2):
                tc_gh = int(TOTCH[g, h])
                idx_flat = np.zeros(tc_gh * P, np.int16)
                # md/mw in [P, NW, KFIX] layout (chunk-minor mask build)
                md = np.zeros((P, NW, KFIX), np.float32)
                mw = np.zeros((P, NW, KFIX), np.float32)
                sel = shf == h
                c = slot[sel] // P
                p = slot[sel] % P
                sw = swin[sel]
                gch = CHOFF[g, h][sw] + c
                idx_flat[gch * P + p] = il[sel].astype(np.int16)
                md[p, sw, c] = dw[sel]
                mw[p, sw, c] = we[sel]
                wr = idx_flat.reshape(-1, 16).T
                im[f"idx{g}{h}"] = np.ascontiguousarray(np.tile(wr, (8, 1)))
                im[f"md{g}{h}"] = md.reshape(P, NW * KFIX).astype(BF16)
                im[f"mw{g}{h}"] = mw.reshape(P, NW * KFIX).astype(BF16)
        in_maps.append(im)
    return in_maps, K, CHOFF, TOTCH


def _build(cfg: Cfg, K, CHOFF, TOTCH):
    G, NW, WIN, WB = cfg.g_num, cfg.nwin, cfg.win, cfg.win_batch
    GRP = cfg.grp
    KC = cfg.kc_cat
    HF = cfg.h_feats
    CLS = cfg.n_classes
    f32, bf16, i16 = mybir.dt.float32, mybir.dt.bfloat16, mybir.dt.int16

    nc = bacc.Bacc(num_swdge_queues=4)
    t_xt = nc.declare_dram_parameter(
        "xtiles", [cfg.ntile_own, P, cfg.kc_in * P], bf16, isOutput=False)
    t_w1 = nc.declare_dram_parameter("w1c", [P, cfg.kc_in * HF], bf16, isOutput=False)
    t_w2 = nc.declare_dram_parameter("w2c", [P, KC * HF], bf16, isOutput=False)
    t_l1w = nc.declare_dram_parameter("l1wc", [P, KC * cfg.cat], bf16, isOutput=False)
    t_l2w = nc.declare_dram_parameter("l2wc", [P, KC * cfg.cat], bf16, isOutput=False)
    t_l3w = nc.declare_dram_parameter("l3wc", [P, KC * CLS], bf16, isOutput=False)
    t_l1b = nc.declare_dram_parameter("l1bc", [P, KC], f32, isOutput=False)
    t_l2b = nc.declare_dram_parameter("l2bc", [P, KC], f32, isOutput=False)
    t_l3b = nc.declare_dram_parameter("l3bc", [CLS, 1], f32, isOutput=False)
    t_iota = nc.declare_dram_parameter("iota", [P, WIN * KFIX], bf16, isOutput=False)
    t_idx, t_md, t_mw = {}, {}, {}
    for g in range(G):
        for h in range(2):
            tc_gh = int(TOTCH[g, h])
            t_idx[(g, h)] = nc.declare_dram_parameter(
                f"idx{g}{h}", [P, tc_gh * 8], i16, isOutput=False)
            t_md[(g, h)] = nc.declare_dram_parameter(
                f"md{g}{h}", [P, NW * KFIX], bf16, isOutput=False)
            t_mw[(g, h)] = nc.declare_dram_parameter(
                f"mw{g}{h}", [P, NW * KFIX], bf16, isOutput=False)
    t_out = nc.declare_dram_parameter("out", [CLS, NW * WIN], f32, isOutput=True)

    d_t1s = nc.dram_tensor("t1s", [cfg.shard_p, HF], bf16)
    d_t1f = nc.dram_tensor("t1f", [cfg.rows, HF], bf16, addr_space="Shared")
    d_t2s = nc.dram_tensor("t2s", [cfg.shard_p, HF], bf16)
    d_t2f = nc.dram_tensor("t2f", [cfg.rows, HF], bf16, addr_space="Shared")

    AF = mybir.ActivationFunctionType
    ALU = mybir.AluOpType
    nb = (NW + WB - 1) // WB
    qctr = [0]
    max_nch = 0
    for g in range(G):
        for h in range(2):
            for b in range(nb):
                w0, w1 = b * WB, min(NW, (b + 1) * WB)
                max_nch = max(max_nch, int(CHOFF[g, h, w1] - CHOFF[g, h, w0]))

    with tile.TileContext(nc) as tc:
        with (
            tc.tile_pool(name="const", bufs=1) as cp,
            tc.tile_pool(name="x", bufs=2) as xp,
            tc.tile_pool(name="gath", bufs=2) as gp,
            tc.tile_pool(name="meta", bufs=2) as mp,
            tc.tile_pool(name="mask", bufs=3) as kp,
            tc.tile_pool(name="hcat", bufs=2) as hp,
            tc.tile_pool(name="dense", bufs=2) as dp,
            tc.tile_pool(name="psa", bufs=2, space="PSUM") as pm,
            tc.tile_pool(name="psw", bufs=2, space="PSUM") as pw,
            tc.tile_pool(name="psb", bufs=2, space="PSUM") as pb,
        ):
            ident = cp.tile([P, P], f32)
            make_identity(nc, ident[:])

            def const_load(t, shape, dtype):
                s = cp.tile(shape, dtype, tag=t.name + "_c")
                nc.sync.dma_start(out=s[:], in_=t[:])
                return s

            w1_sb = const_load(t_w1, [P, cfg.kc_in * HF], bf16)
            w2_sb = const_load(t_w2, [P, KC * HF], bf16)
            l1w_sb = const_load(t_l1w, [P, KC * cfg.cat], bf16)
            l2w_sb = const_load(t_l2w, [P, KC * cfg.cat], bf16)
            l3w_sb = const_load(t_l3w, [P, KC * CLS], bf16)
            l1b_sb = const_load(t_l1b, [P, KC], f32)
            l2b_sb = const_load(t_l2b, [P, KC], f32)
            l3b_sb = const_load(t_l3b, [CLS, 1], f32)
            iota_sb = const_load(t_iota, [P, WIN * KFIX], bf16)
            out_sb = cp.tile([CLS, NW * WIN], f32)

            # ------------- SpMM + dense layers, per window batch -------------
            def load_meta(g, h, b):
                w0 = b * WB
                w1 = min(NW, w0 + WB)
                nwb = w1 - w0
                c0 = int(CHOFF[g, h, w0])
                c1 = int(CHOFF[g, h, w1])
                nch = c1 - c0
                if nch == 0:
                    return None
                idx_t = mp.tile([P, max_nch * 8], i16, tag=f"idx{h}",
                                name=f"idx{g}{h}{b}")
                nc.sync.dma_start(out=idx_t[:, :nch * 8],
                                  in_=t_idx[(g, h)][:, c0 * 8:c1 * 8])
                md_t = mp.tile([P, WB * KFIX], bf16, tag=f"md{h}",
                               name=f"md{g}{h}{b}")
                nc.sync.dma_start(out=md_t[:, :nwb * KFIX],
                                  in_=t_md[(g, h)][:, w0 * KFIX:w1 * KFIX])
                mw_t = mp.tile([P, WB * KFIX], bf16, tag=f"mw{h}",
                               name=f"mw{g}{h}{b}")
                nc.sync.dma_start(out=mw_t[:, :nwb * KFIX],
                                  in_=t_mw[(g, h)][:, w0 * KFIX:w1 * KFIX])
                return idx_t, md_t, mw_t

            def spmm_layer(table, layer2):
                for b in range(nb):
                    w0 = b * WB
                    w1 = min(NW, w0 + WB)
                    nwb = w1 - w0
                    fts, mds, mws = {}, {}, {}
                    hcat = {}
                    for g in range(G):
                        hcat[g] = hp.tile([P, WB * WIN], bf16, tag=f"hc{g}",
                                          name=f"hc{g}")
                    for g in range(G):
                        for h in range(2):
                            c0 = int(CHOFF[g, h, w0])
                            c1 = int(CHOFF[g, h, w1])
                            nch = c1 - c0
                            if nch == 0:
                                continue
                            meta = load_meta(g, h, b)
                            idx_t, md_t, mw_t = meta
                            ft = gp.tile([P, max_nch * HF], bf16, tag=f"ft{h}")
                            GCH = 8
                            for j in range(0, nch, GCH):
                                gl = min(GCH, nch - j)
                                ni = gl * P
                                nc.gpsimd.dma_gather(
                                    out_ap=ft[:, j * HF:(j + gl) * HF]
                                    .rearrange("p (k f) -> p k f", f=HF),
                                    in_ap=table[(cfg.half if h else 0):
                                                (cfg.rows if h else cfg.half),
                                                :],
                                    idxs_ap=idx_t[:, j * 8:(j + gl) * 8],
                                    num_idxs=ni, num_idxs_reg=ni,
                                    elem_size=HF, elem_step=HF,
                                    queue_num=qctr[0] % 4,
                                )
                                qctr[0] += 1
                            fts[(g, h)] = ft
                            mds[(g, h)] = md_t
                            mws[(g, h)] = mw_t
                        # aggregate windows of this batch for graph g
                        for wi in range(w0, w1):
                            kw = int(K[g, wi, 0] + K[g, wi, 1])
                            if kw == 0:
                                continue
                            ps = pm.tile([P, WIN], f32, tag="agg")
                            ci = 0
                            for h in range(2):
                                kh = int(K[g, wi, h])
                                if kh == 0:
                                    continue
                                # bulk 2x mask build: [P, WIN, KFIX] region
                                mk = kp.tile([P, WIN * KFIX], bf16, tag="mk")
                                md_t, mw_t = mds[(g, h)], mws[(g, h)]
                                dwc = wi - w0
                                md_ap = md_t[:, dwc * KFIX:(dwc + 1) * KFIX]
                                mw_ap = mw_t[:, dwc * KFIX:(dwc + 1) * KFIX]
                                mk3 = mk[:].rearrange("p (j c) -> p j c",
                                                      c=KFIX)
                                nc.vector.tensor_tensor(
                                    out=mk3,
                                    in0=bass.AP(md_ap.tensor, md_ap.offset,
                                                [list(md_ap.ap[0]), [0, WIN],
                                                 list(md_ap.ap[1])]),
                                    in1=iota_sb[:].rearrange(
                                        "p (j c) -> p j c", c=KFIX),
                                    op=ALU.is_equal)
                                nc.vector.tensor_tensor(
                                    out=mk3, in0=mk3,
                                    in1=bass.AP(mw_ap.tensor, mw_ap.offset,
                                                [list(mw_ap.ap[0]), [0, WIN],
                                                 list(mw_ap.ap[1])]),
                                    op=ALU.mult)
                                cw0 = int(CHOFF[g, h, wi]) - int(
                                    CHOFF[g, h, w0])
                                ft = fts[(g, h)]
                                for c in range(kh):
                                    cc = cw0 + c
                                    nc.tensor.matmul(
                                        out=ps[:],
                                        lhsT=ft[:, cc * HF:(cc + 1) * HF],
                                        rhs=mk3[:, :, c],
                                        start=(ci == 0), stop=(ci == kw - 1))
                                    ci += 1
                            dwc = wi - w0
                            nc.scalar.activation(
                                hcat[g][:, dwc * WIN:(dwc + 1) * WIN],
                                ps[:], AF.Relu)
                    # dense layers per 4-window group
                    for g0 in range(w0, w1, GRP):
                        g1 = min(w1, g0 + GRP)
                        ncol = (g1 - g0) * WIN
                        s0 = (g0 - w0) * WIN
                        if not layer2:
                            def mlp(ws, bs, ins, ins_off, name):
                                outs = []
                                for fc in range(KC):
                                    ps = pw.tile([P, GRP * WIN], f32,
                                                 tag="mlp")
                                    for kc in range(KC):
                                        nc.tensor.matmul(
                                            out=ps[:, :ncol],
                                            lhsT=ws[:, (kc * KC + fc) * P:
                                                    (kc * KC + fc + 1) * P],
                                            rhs=ins[kc][:, ins_off:
                                                        ins_off + ncol],
                                            start=(kc == 0),
                                            stop=(kc == KC - 1))
                                    o = dp.tile([P, GRP * WIN], bf16,
                                                tag=f"mlpo{name}{fc}")
                                    nc.scalar.activation(
                                        o[:, :ncol], ps[:, :ncol], AF.Relu,
                                        bias=bs[:, fc:fc + 1])
                                    outs.append(o)
                                return outs
                            hl1 = mlp(l1w_sb, l1b_sb,
                                      [hcat[g] for g in range(G)], s0, "a")
                            hl2 = mlp(l2w_sb, l2b_sb, hl1, 0, "b")
                            p2 = pw.tile([P, GRP * WIN], f32, tag="mlp")
                            for kc in range(KC):
                                nc.tensor.matmul(
                                    out=p2[:, :ncol],
                                    lhsT=w2_sb[:, kc * HF:(kc + 1) * HF],
                                    rhs=hl2[kc][:, :ncol],
                                    start=(kc == 0), stop=(kc == KC - 1))
                            p2s = dp.tile([P, GRP * WIN], f32, tag="p2s")
                            nc.scalar.activation(p2s[:, :ncol], p2[:, :ncol],
                                                 AF.Copy)
                            for wi in range(g0, g1):
                                co = (wi - g0) * WIN
                                p2t = pb.tile([WIN, P], f32, tag="misc")
                                nc.tensor.transpose(
                                    p2t[:], p2s[:, co:co + WIN], ident[:])
                                h2r = dp.tile([WIN, HF], bf16, tag="h2r")
                                nc.scalar.activation(h2r[:], p2t[:], AF.Copy)
                                nc.sync.dma_start(
                                    out=d_t2s[wi * WIN:(wi + 1) * WIN, :],
                                    in_=h2r[:])
                        else:
                            ps = pw.tile([CLS, GRP * WIN], f32, tag="mlp")
                            for kc in range(KC):
                                nc.tensor.matmul(
                                    out=ps[:, :ncol],
                                    lhsT=l3w_sb[:, kc * CLS:(kc + 1) * CLS],
                                    rhs=hcat[kc][:, s0:s0 + ncol],
                                    start=(kc == 0), stop=(kc == KC - 1))
                            nc.vector.tensor_scalar(
                                out=out_sb[:, g0 * WIN:g0 * WIN + ncol],
                                in0=ps[:, :ncol], scalar1=l3b_sb[:],
                                scalar2=None, op0=ALU.add)

            # ---------------- phase 1: own-shard T1 = x @ W1 ----------------
            for t in range(cfg.ntile_own):
                xt = xp.tile([P, cfg.kc_in * P], bf16, tag="xt")
                nc.sync.dma_start(out=xt[:], in_=t_xt[t])
                q1 = pb.tile([P, HF], f32, tag="misc")
                for kc in range(cfg.kc_in):
                    nc.tensor.matmul(
                        out=q1[:], lhsT=xt[:, kc * P:(kc + 1) * P],
                        rhs=w1_sb[:, kc * HF:(kc + 1) * HF],
                        start=(kc == 0), stop=(kc == cfg.kc_in - 1))
                h1 = xp.tile([P, HF], bf16, tag="h1")
                nc.scalar.activation(h1[:], q1[:], AF.Copy)
                nc.sync.dma_start(out=d_t1s[t * P:(t + 1) * P, :], in_=h1[:])

            tc.strict_bb_all_engine_barrier()
            nc.gpsimd.collective_compute(
                "AllGather", mybir.AluOpType.bypass,
                ins=[d_t1s[:]], outs=[d_t1f[:]],
                replica_groups=[list(range(cfg.n_cores))],
            )
            tc.strict_bb_all_engine_barrier()

            spmm_layer(d_t1f, layer2=False)

            tc.strict_bb_all_engine_barrier()
            nc.gpsimd.collective_compute(
                "AllGather", mybir.AluOpType.bypass,
                ins=[d_t2s[:]], outs=[d_t2f[:]],
                replica_groups=[list(range(cfg.n_cores))],
            )
            tc.strict_bb_all_engine_barrier()

            spmm_layer(d_t2f, layer2=True)

            nc.sync.dma_start(out=t_out[:], in_=out_sb[:])
    nc.finalize()
    return nc


def _run(cfg: Cfg, inputs: dict, trace: bool = False):
    _install_ntff_hook()
    from concourse import bass_utils
    bass_utils.upload_artifacts = lambda d: "local://skipped"
    from concourse.bass_utils import run_bass_kernel_spmd

    in_maps, K, CHOFF, TOTCH = _prep_inputs(cfg, **inputs)
    nc = _build(cfg, K, CHOFF, TOTCH)
    res = run_bass_kernel_spmd(nc, in_maps, list(range(cfg.n_cores)),
                               trace=trace)
    outs = []
    for i in range(cfg.n_cores):
        o = res.results[i]["out"]                   # [CLS, nwin*win]
        outs.append(o.T[:cfg.shard])                # [shard, CLS]
    full = np.concatenate(outs, axis=0)
    return full, res.exec_time_ns


def kernel(**inputs) -> np.ndarray:
    cfg = Cfg()
    out, _ = _run(cfg, inputs, trace=False)
    return out.astype(np.float32)


# revision 21
# speedup vs baseline: 1.0453x; 1.0453x over previous
"""Trainium2 Bass kernel for the GWNN2 GNN (4-graph GraphConv x2 + MLP).

V3 strategy (8 NeuronCores, dst-sharded):
  * deg_out[src] and deg_in[dst] both folded into the per-edge weight
    host-side -> each layer's gather table is ONE shared [50176, 128]
    bf16 matrix (q1 = x@W1, q2 = h2@W2); 4x smaller collectives.
  * per-(g, win, half) chunk counts K exact (max over the 8 cores so one
    SPMD NEFF serves all); dma_gather in 8-chunk (1024-desc) sub-calls
    (the SWDGE ring caps a call at ~1024 descriptors).
  * one-hot masks built in chunk-MINOR layout [e, win, c] with two bulk
    DVE tensor_tensor ops per (g, half, win) whose operands are all
    2-byte packed (DVE 2x mode): (md == iota) * w.  The aggregation
    matmul reads the mask chunk column with a strided rhs AP.
  * dense layers (l1, l2, W2 / l3) batched over 4-window groups
    (rhs 512 wide) to amortize PE ldweights.
  * AllGather shard tables between phases.
"""
import sys
import types
from dataclasses import dataclass

if "/opt/trn_rl_repo" not in sys.path:
    sys.path.insert(0, "/opt/trn_rl_repo")

import numpy as np
import ml_dtypes

import concourse.bass as bass
import concourse.bacc as bacc
import concourse.mybir as mybir
import concourse.tile as tile
from concourse.masks import make_identity

BF16 = ml_dtypes.bfloat16
P = 128
KFIX = 10            # mask layout c-pitch (>= max chunks per bucket)


def _install_ntff_hook():
    """Make trace=True usable under axon (antenv.axon_hooks may be absent)."""
    try:
        import antenv
        if "antenv.axon_hooks" in sys.modules:
            return
        m = types.ModuleType("antenv.axon_hooks")
        box = [None]
        m.set_axon_ntff_profile_hook = lambda h: box.__setitem__(0, h)
        m.get_axon_ntff_profile_hook = lambda: box[0]
        sys.modules["antenv.axon_hooks"] = m
        antenv.axon_hooks = m
        try:
            from trn_agent_boot.trn_boot import _ntff_profile_via_ctypes
            hook = _ntff_profile_via_ctypes("/opt/axon/libaxon_pjrt.so")
            if hook is not None:
                m.set_axon_ntff_profile_hook(hook)
        except Exception:
            pass
    except Exception:
        pass


@dataclass
class Cfg:
    n_nodes: int = 50000
    g_num: int = 4
    in_feats: int = 256
    h_feats: int = 128
    n_classes: int = 40
    n_cores: int = 8
    win: int = 128
    win_batch: int = 8          # windows per ft/gather batch
    grp: int = 4                # windows per dense-layer matmul group

    @property
    def shard(self):
        return self.n_nodes // self.n_cores

    @property
    def shard_p(self):
        return ((self.shard + P - 1) // P) * P

    @property
    def rows(self):
        return self.shard_p * self.n_cores

    @property
    def half(self):
        return self.rows // 2

    @property
    def nwin(self):
        return self.shard_p // self.win

    @property
    def cat(self):
        return self.h_feats * self.g_num

    @property
    def kc_cat(self):
        return self.cat // P

    @property
    def kc_in(self):
        return self.in_feats // P

    @property
    def ntile_own(self):
        return self.shard_p // P


def _prep_inputs(cfg: Cfg, in_feat, src, dst, w, W1, W2, l1w, l1b, l2w, l2b,
                 l3w, l3b):
    """Host-side sharding/packing."""
    N, G = cfg.n_nodes, cfg.g_num
    SH, SHP = cfg.shard, cfg.shard_p
    NW, WIN = cfg.nwin, cfg.win
    HALF = cfg.half
    src = np.asarray(src).astype(np.int64)
    dst = np.asarray(dst).astype(np.int64)
    w = np.asarray(w, dtype=np.float32)
    in_feat = np.asarray(in_feat, dtype=np.float32)

    deg_out = np.empty((G, N), np.float32)
    deg_in = np.empty((G, N), np.float32)
    for g in range(G):
        deg_out[g] = np.clip(np.bincount(src[g], minlength=N), 1.0, None) ** -0.5
        deg_in[g] = np.clip(np.bincount(dst[g], minlength=N), 1.0, None) ** -0.5

    src_pad = (src // SH) * SHP + (src % SH)
    half_flag = (src_pad >= HALF).astype(np.int64)
    idx_local = (src_pad - half_flag * HALF).astype(np.int64)

    core_of = dst // SH
    dst_loc = dst % SH
    win_of = dst_loc // WIN
    dst_in_win = (dst_loc % WIN).astype(np.float32)

    w_eff = np.empty((G, src.shape[1]), np.float32)
    for g in range(G):
        w_eff[g] = w[g] * deg_in[g][dst[g]] * deg_out[g][src[g]]

    cnt = np.zeros((cfg.n_cores, G, NW, 2), np.int64)
    for i in range(cfg.n_cores):
        for g in range(G):
            m = core_of[g] == i
            key = win_of[g][m] * 2 + half_flag[g][m]
            cnt[i, g] = np.bincount(key, minlength=NW * 2).reshape(NW, 2)
    K = np.ceil(cnt.max(axis=0) / P).astype(np.int64)       # (G, NW, 2)
    assert K.max() <= KFIX, f"KFIX too small: {K.max()}"
    CHOFF = np.zeros((G, 2, NW + 1), np.int64)
    for g in range(G):
        for h in range(2):
            CHOFF[g, h, 1:] = np.cumsum(K[g, :, h])
    TOTCH = CHOFF[:, :, -1]                                  # (G, 2)

    xpad = np.zeros((cfg.rows, cfg.in_feats), np.float32)
    for i in range(cfg.n_cores):
        xpad[i * SHP:i * SHP + SH] = in_feat[i * SH:(i + 1) * SH]

    def pack_lhsT(W, kc):
        Wr = np.asarray(W, np.float32).reshape(kc, P, -1)
        return np.ascontiguousarray(Wr.transpose(1, 0, 2)).reshape(P, -1)

    W1c = pack_lhsT(W1, cfg.kc_in).astype(BF16)
    W2c = pack_lhsT(W2, cfg.kc_cat).astype(BF16)
    l1wc = pack_lhsT(l1w, cfg.kc_cat).astype(BF16)
    l2wc = pack_lhsT(l2w, cfg.kc_cat).astype(BF16)
    l3wc = pack_lhsT(l3w, cfg.kc_cat).astype(BF16)
    l1bc = np.ascontiguousarray(
        np.asarray(l1b, np.float32).reshape(cfg.kc_cat, P).T)
    l2bc = np.ascontiguousarray(
        np.asarray(l2b, np.float32).reshape(cfg.kc_cat, P).T)
    l3bc = np.asarray(l3b, np.float32).reshape(cfg.n_classes, 1)

    # iota tile for mask build: [128, WIN, KFIX] bf16, value = j
    iota_t = np.ascontiguousarray(
        np.broadcast_to(np.arange(WIN, dtype=np.float32)[None, :, None],
                        (P, WIN, KFIX))).reshape(P, WIN * KFIX).astype(BF16)

    in_maps = []
    for i in range(cfg.n_cores):
        im = {"w1c": W1c, "w2c": W2c, "l1wc": l1wc, "l2wc": l2wc,
              "l3wc": l3wc, "l1bc": l1bc, "l2bc": l2bc, "l3bc": l3bc,
              "iota": iota_t}
        xsh = xpad[i * SHP:(i + 1) * SHP].reshape(
            cfg.ntile_own, P, cfg.kc_in, P)
        im["xtiles"] = np.ascontiguousarray(
            xsh.transpose(0, 3, 2, 1)).reshape(
            cfg.ntile_own, P, cfg.kc_in * P).astype(BF16)

        for g in range(G):
            m = core_of[g] == i
            key = win_of[g][m] * 2 + half_flag[g][m]
            order = np.argsort(key, kind="stable")
            skey = key[order]
            bc = np.bincount(skey, minlength=NW * 2)
            starts = np.concatenate([[0], np.cumsum(bc)[:-1]])
            slot = np.arange(len(skey)) - starts[skey]
            il = idx_local[g][m][order]
            dw = dst_in_win[g][m][order]
            we = w_eff[g][m][order]
            swin = skey // 2
            shf = skey % 2
            for h in range(2):
                tc_gh = int(TOTCH[g, h])
                idx_flat = np.zeros(tc_gh * P, np.int16)
                # md/mw in [P, NW, KFIX] layout (chunk-minor mask build)
                md = np.zeros((P, NW, KFIX), np.float32)
                mw = np.zeros((P, NW, KFIX), np.float32)
                sel = shf == h
                c = slot[sel] // P
                p = slot[sel] % P
                sw = swin[sel]
                gch = CHOFF[g, h][sw] + c
                idx_flat[gch * P + p] = il[sel].astype(np.int16)
                md[p, sw, c] = dw[sel]
                mw[p, sw, c] = we[sel]
                wr = idx_flat.reshape(-1, 16).T
                im[f"idx{g}{h}"] = np.ascontiguousarray(np.tile(wr, (8, 1)))
                im[f"md{g}{h}"] = md.reshape(P, NW * KFIX).astype(BF16)
                im[f"mw{g}{h}"] = mw.reshape(P, NW * KFIX).astype(BF16)
        in_maps.append(im)
    return in_maps, K, CHOFF, TOTCH


def _build(cfg: Cfg, K, CHOFF, TOTCH):
    G, NW, WIN, WB = cfg.g_num, cfg.nwin, cfg.win, cfg.win_batch
    GRP = cfg.grp
    KC = cfg.kc_cat
    HF = cfg.h_feats
    CLS = cfg.n_classes
    f32, bf16, i16 = mybir.dt.float32, mybir.dt.bfloat16, mybir.dt.int16

    nc = bacc.Bacc(num_swdge_queues=4)
    t_xt = nc.declare_dram_parameter(
        "xtiles", [cfg.ntile_own, P, cfg.kc_in * P], bf16, isOutput=False)
    t_w1 = nc.declare_dram_parameter("w1c", [P, cfg.kc_in * HF], bf16, isOutput=False)
    t_w2 = nc.declare_dram_parameter("w2c", [P, KC * HF], bf16, isOutput=False)
    t_l1w = nc.declare_dram_parameter("l1wc", [P, KC * cfg.cat], bf16, isOutput=False)
    t_l2w = nc.declare_dram_parameter("l2wc", [P, KC * cfg.cat], bf16, isOutput=False)
    t_l3w = nc.declare_dram_parameter("l3wc", [P, KC * CLS], bf16, isOutput=False)
    t_l1b = nc.declare_dram_parameter("l1bc", [P, KC], f32, isOutput=False)
    t_l2b = nc.declare_dram_parameter("l2bc", [P, KC], f32, isOutput=False)
    t_l3b = nc.declare_dram_parameter("l3bc", [CLS, 1], f32, isOutput=False)
    t_iota = nc.declare_dram_parameter("iota", [P, WIN * KFIX], bf16, isOutput=False)
    t_idx, t_md, t_mw = {}, {}, {}
    for g in range(G):
        for h in range(2):
            tc_gh = int(TOTCH[g, h])
            t_idx[(g, h)] = nc.declare_dram_parameter(
                f"idx{g}{h}", [P, tc_gh * 8], i16, isOutput=False)
            t_md[(g, h)] = nc.declare_dram_parameter(
                f"md{g}{h}", [P, NW * KFIX], bf16, isOutput=False)
            t_mw[(g, h)] = nc.declare_dram_parameter(
                f"mw{g}{h}", [P, NW * KFIX], bf16, isOutput=False)
    t_out = nc.declare_dram_parameter("out", [CLS, NW * WIN], f32, isOutput=True)

    d_t1s = nc.dram_tensor("t1s", [cfg.shard_p, HF], bf16)
    d_t1f = nc.dram_tensor("t1f", [cfg.rows, HF], bf16, addr_space="Shared")
    d_t2s = nc.dram_tensor("t2s", [cfg.shard_p, HF], bf16)
    d_t2f = nc.dram_tensor("t2f", [cfg.rows, HF], bf16, addr_space="Shared")

    AF = mybir.ActivationFunctionType
    ALU = mybir.AluOpType
    nb = (NW + WB - 1) // WB
    qctr = [0]
    max_nch = 0
    for g in range(G):
        for h in range(2):
            for b in range(nb):
                w0, w1 = b * WB, min(NW, (b + 1) * WB)
                max_nch = max(max_nch, int(CHOFF[g, h, w1] - CHOFF[g, h, w0]))

    with tile.TileContext(nc) as tc:
        with (
            tc.tile_pool(name="const", bufs=1) as cp,
            tc.tile_pool(name="x", bufs=3) as xp,
            tc.tile_pool(name="gath", bufs=3) as gp,
            tc.tile_pool(name="meta", bufs=2) as mp,
            tc.tile_pool(name="mask", bufs=3) as kp,
            tc.tile_pool(name="hcat", bufs=3) as hp,
            tc.tile_pool(name="dense", bufs=2) as dp,
            tc.tile_pool(name="psa", bufs=2, space="PSUM") as pm,
            tc.tile_pool(name="psw", bufs=2, space="PSUM") as pw,
            tc.tile_pool(name="psb", bufs=2, space="PSUM") as pb,
        ):
            ident = cp.tile([P, P], f32)
            make_identity(nc, ident[:])

            def const_load(t, shape, dtype):
                s = cp.tile(shape, dtype, tag=t.name + "_c")
                nc.sync.dma_start(out=s[:], in_=t[:])
                return s

            w1_sb = const_load(t_w1, [P, cfg.kc_in * HF], bf16)
            w2_sb = const_load(t_w2, [P, KC * HF], bf16)
            l1w_sb = const_load(t_l1w, [P, KC * cfg.cat], bf16)
            l2w_sb = const_load(t_l2w, [P, KC * cfg.cat], bf16)
            l3w_sb = const_load(t_l3w, [P, KC * CLS], bf16)
            l1b_sb = const_load(t_l1b, [P, KC], f32)
            l2b_sb = const_load(t_l2b, [P, KC], f32)
            l3b_sb = const_load(t_l3b, [CLS, 1], f32)
            iota_sb = const_load(t_iota, [P, WIN * KFIX], bf16)
            out_sb = cp.tile([CLS, NW * WIN], f32)

            # ------------- SpMM + dense layers, per window batch -------------
            def load_meta(g, h, b):
                w0 = b * WB
                w1 = min(NW, w0 + WB)
                nwb = w1 - w0
                c0 = int(CHOFF[g, h, w0])
                c1 = int(CHOFF[g, h, w1])
                nch = c1 - c0
                if nch == 0:
                    return None
                idx_t = mp.tile([P, max_nch * 8], i16, tag=f"idx{h}",
                                name=f"idx{g}{h}{b}")
                nc.sync.dma_start(out=idx_t[:, :nch * 8],
                                  in_=t_idx[(g, h)][:, c0 * 8:c1 * 8])
                md_t = mp.tile([P, WB * KFIX], bf16, tag=f"md{h}",
                               name=f"md{g}{h}{b}")
                nc.sync.dma_start(out=md_t[:, :nwb * KFIX],
                                  in_=t_md[(g, h)][:, w0 * KFIX:w1 * KFIX])
                mw_t = mp.tile([P, WB * KFIX], bf16, tag=f"mw{h}",
                               name=f"mw{g}{h}{b}")
                nc.sync.dma_start(out=mw_t[:, :nwb * KFIX],
                                  in_=t_mw[(g, h)][:, w0 * KFIX:w1 * KFIX])
                return idx_t, md_t, mw_t

            def spmm_layer(table, layer2):
                for b in range(nb):
                    w0 = b * WB
                    w1 = min(NW, w0 + WB)
                    nwb = w1 - w0
                    fts, mds, mws = {}, {}, {}
                    hcat = {}
                    for g in range(G):
                        hcat[g] = hp.tile([P, WB * WIN], bf16, tag=f"hc{g}",
                                          name=f"hc{g}")
                    for g in range(G):
                        for h in range(2):
                            c0 = int(CHOFF[g, h, w0])
                            c1 = int(CHOFF[g, h, w1])
                            nch = c1 - c0
                            if nch == 0:
                                continue
                            meta = load_meta(g, h, b)
                            idx_t, md_t, mw_t = meta
                            ft = gp.tile([P, max_nch * HF], bf16, tag=f"ft{h}")
                            GCH = 8
                            for j in range(0, nch, GCH):
                                gl = min(GCH, nch - j)
                                ni = gl * P
                                nc.gpsimd.dma_gather(
                                    out_ap=ft[:, j * HF:(j + gl) * HF]
                                    .rearrange("p (k f) -> p k f", f=HF),
                                    in_ap=table[(cfg.half if h else 0):
                                                (cfg.rows if h else cfg.half),
                                                :],
                                    idxs_ap=idx_t[:, j * 8:(j + gl) * 8],
                                    num_idxs=ni, num_idxs_reg=ni,
                                    elem_size=HF, elem_step=HF,
                                    queue_num=qctr[0] % 4,
                                )
                                qctr[0] += 1
                            fts[(g, h)] = ft
                            mds[(g, h)] = md_t
                            mws[(g, h)] = mw_t
                        # aggregate windows of this batch for graph g
                        for wi in range(w0, w1):
                            kw = int(K[g, wi, 0] + K[g, wi, 1])
                            if kw == 0:
                                continue
                            ps = pm.tile([P, WIN], f32, tag="agg")
                            ci = 0
                            for h in range(2):
                                kh = int(K[g, wi, h])
                                if kh == 0:
                                    continue
                                # bulk 2x mask build: [P, WIN, KFIX] region
                                mk = kp.tile([P, WIN * KFIX], bf16, tag="mk")
                                md_t, mw_t = mds[(g, h)], mws[(g, h)]
                                dwc = wi - w0
                                md_ap = md_t[:, dwc * KFIX:(dwc + 1) * KFIX]
                                mw_ap = mw_t[:, dwc * KFIX:(dwc + 1) * KFIX]
                                mk3 = mk[:].rearrange("p (j c) -> p j c",
                                                      c=KFIX)
                                nc.vector.tensor_tensor(
                                    out=mk3,
                                    in0=bass.AP(md_ap.tensor, md_ap.offset,
                                                [list(md_ap.ap[0]), [0, WIN],
                                                 list(md_ap.ap[1])]),
                                    in1=iota_sb[:].rearrange(
                                        "p (j c) -> p j c", c=KFIX),
                                    op=ALU.is_equal)
                                nc.vector.tensor_tensor(
                                    out=mk3, in0=mk3,
                                    in1=bass.AP(mw_ap.tensor, mw_ap.offset,
                                                [list(mw_ap.ap[0]), [0, WIN],
                                                 list(mw_ap.ap[1])]),
                                    op=ALU.mult)
                                cw0 = int(CHOFF[g, h, wi]) - int(
                                    CHOFF[g, h, w0])
                                ft = fts[(g, h)]
                                for c in range(kh):
                                    cc = cw0 + c
                                    nc.tensor.matmul(
                                        out=ps[:],
                                        lhsT=ft[:, cc * HF:(cc + 1) * HF],
                                        rhs=mk3[:, :, c],
                                        start=(ci == 0), stop=(ci == kw - 1))
                                    ci += 1
                            dwc = wi - w0
                            nc.scalar.activation(
                                hcat[g][:, dwc * WIN:(dwc + 1) * WIN],
                                ps[:], AF.Relu)
                    # dense layers per 4-window group
                    for g0 in range(w0, w1, GRP):
                        g1 = min(w1, g0 + GRP)
                        ncol = (g1 - g0) * WIN
                        s0 = (g0 - w0) * WIN
                        if not layer2:
                            def mlp(ws, bs, ins, ins_off, name):
                                outs = []
                                for fc in range(KC):
                                    ps = pw.tile([P, GRP * WIN], f32,
                                                 tag="mlp")
                                    for kc in range(KC):
                                        nc.tensor.matmul(
                                            out=ps[:, :ncol],
                                            lhsT=ws[:, (kc * KC + fc) * P:
                                                    (kc * KC + fc + 1) * P],
                                            rhs=ins[kc][:, ins_off:
                                                        ins_off + ncol],
                                            start=(kc == 0),
                                            stop=(kc == KC - 1))
                                    o = dp.tile([P, GRP * WIN], bf16,
                                                tag=f"mlpo{name}{fc}")
                                    nc.scalar.activation(
                                        o[:, :ncol], ps[:, :ncol], AF.Relu,
                                        bias=bs[:, fc:fc + 1])
                                    outs.append(o)
                                return outs
                            hl1 = mlp(l1w_sb, l1b_sb,
                                      [hcat[g] for g in range(G)], s0, "a")
                            hl2 = mlp(l2w_sb, l2b_sb, hl1, 0, "b")
                            p2 = pw.tile([P, GRP * WIN], f32, tag="mlp")
                            for kc in range(KC):
                                nc.tensor.matmul(
                                    out=p2[:, :ncol],
                                    lhsT=w2_sb[:, kc * HF:(kc + 1) * HF],
                                    rhs=hl2[kc][:, :ncol],
                                    start=(kc == 0), stop=(kc == KC - 1))
                            p2s = dp.tile([P, GRP * WIN], f32, tag="p2s")
                            nc.scalar.activation(p2s[:, :ncol], p2[:, :ncol],
                                                 AF.Copy)
                            for wi in range(g0, g1):
                                co = (wi - g0) * WIN
                                p2t = pb.tile([WIN, P], f32, tag="misc")
                                nc.tensor.transpose(
                                    p2t[:], p2s[:, co:co + WIN], ident[:])
                                h2r = dp.tile([WIN, HF], bf16, tag="h2r")
                                nc.scalar.activation(h2r[:], p2t[:], AF.Copy)
                                nc.sync.dma_start(
                                    out=d_t2s[wi * WIN:(wi + 1) * WIN, :],
                                    in_=h2r[:])
                        else:
                            ps = pw.tile([CLS, GRP * WIN], f32, tag="mlp")
                            for kc in range(KC):
                                nc.tensor.matmul(
                                    out=ps[:, :ncol],
                                    lhsT=l3w_sb[:, kc * CLS:(kc + 1) * CLS],
                                    rhs=hcat[kc][:, s0:s0 + ncol],
                                    start=(kc == 0), stop=(kc == KC - 1))
                            nc.vector.tensor_scalar(
                                out=out_sb[:, g0 * WIN:g0 * WIN + ncol],
                                in0=ps[:, :ncol], scalar1=l3b_sb[:],
                                scalar2=None, op0=ALU.add)

            # ---------------- phase 1: own-shard T1 = x @ W1 ----------------
            for t in range(cfg.ntile_own):
                xt = xp.tile([P, cfg.kc_in * P], bf16, tag="xt")
                nc.sync.dma_start(out=xt[:], in_=t_xt[t])
                q1 = pb.tile([P, HF], f32, tag="misc")
                for kc in range(cfg.kc_in):
                    nc.tensor.matmul(
                        out=q1[:], lhsT=xt[:, kc * P:(kc + 1) * P],
                        rhs=w1_sb[:, kc * HF:(kc + 1) * HF],
                        start=(kc == 0), stop=(kc == cfg.kc_in - 1))
                h1 = xp.tile([P, HF], bf16, tag="h1")
                nc.scalar.activation(h1[:], q1[:], AF.Copy)
                nc.sync.dma_start(out=d_t1s[t * P:(t + 1) * P, :], in_=h1[:])

            tc.strict_bb_all_engine_barrier()
            nc.gpsimd.collective_compute(
                "AllGather", mybir.AluOpType.bypass,
                ins=[d_t1s[:]], outs=[d_t1f[:]],
                replica_groups=[list(range(cfg.n_cores))],
            )
            tc.strict_bb_all_engine_barrier()

            spmm_layer(d_t1f, layer2=False)

            tc.strict_bb_all_engine_barrier()
            nc.gpsimd.collective_compute(
                "AllGather", mybir.AluOpType.bypass,
                ins=[d_t2s[:]], outs=[d_t2f[:]],
                replica_groups=[list(range(cfg.n_cores))],
            )
            tc.strict_bb_all_engine_barrier()

            spmm_layer(d_t2f, layer2=True)

            nc.sync.dma_start(out=t_out[:], in_=out_sb[:])
    nc.finalize()
    return nc


def _run(cfg: Cfg, inputs: dict, trace: bool = False):
    _install_ntff_hook()
    from concourse import bass_utils
    bass_utils.upload_artifacts = lambda d: "local://skipped"
    from concourse.bass_utils import run_bass_kernel_spmd

    in_maps, K, CHOFF, TOTCH = _prep_inputs(cfg, **inputs)
    nc = _build(cfg, K, CHOFF, TOTCH)
    res = run_bass_kernel_spmd(nc, in_maps, list(range(cfg.n_cores)),
                               trace=trace)
    outs = []
    for i in range(cfg.n_cores):
        o = res.results[i]["out"]                   # [CLS, nwin*win]
        outs.append(o.T[:cfg.shard])                # [shard, CLS]
    full = np.concatenate(outs, axis=0)
    return full, res.exec_time_ns


def kernel(**inputs) -> np.ndarray:
    cfg = Cfg()
    out, _ = _run(cfg, inputs, trace=False)
    return out.astype(np.float32)
